# revision 1
# baseline (speedup 1.0000x reference)
"""DampedLinOSSLayer Trainium2 kernel (8 NeuronCores, batch-sharded).

Math: per SSM channel p, the complex diagonal recurrence
    x_t = lam_p * x_{t-1} + bu_t,   lam_p = r_p * exp(i*th_p)
is factored through the gauge x_t = exp(i*th_p*t) * y_t:
    y_t = r_p * y_{t-1} + c_t,      c_t = exp(-i*th_p*t) * bu_t
which has a REAL per-channel coefficient -> runs as hardware
tensor_tensor_scan (DVE) on the re/im planes independently.
The phase rotations exp(-/+ i*th*t) are split as t = 512*T + t0:
the chunk part exp(+-i*th*512T) is folded (on host) into per-chunk
copies of the B / C projection weights; only the in-chunk part
exp(+-i*th*t0), t0 in [0,512), is applied on-device as elementwise
multiplies with constant [128, 512] tables.

Layout on device ("ST-form"): SSM channel p on partitions (2 halves of
128), time on the free dim. Per core: 4 batches of the 32.
  - input tiles [l,h] -> PE transpose -> inT [h, l]
  - B-proj:  bu[p_half, t]  = B_J^T.T @ inT        (PE)
  - pre-rotation (packed complex mul)              (DVE)
  - scan y = r*y + c along t, full L=2048          (DVE tensor_tensor_scan)
  - post-rotation -> x                             (DVE)
  - C-proj + D-residual -> out[t, h]               (PE, PSUM-accumulated)
"""

import functools
import numpy as np

BATCH, LENGTH, HIDDEN, P = 32, 2048, 128, 256
N_CORES = 8
BPC = BATCH // N_CORES          # batches per core
CH = 512                        # chunk size (phase fold granularity)
NCH = LENGTH // CH              # 4 chunks
NBLK = CH // 128                # 4 token-blocks of 128 per chunk

_COMPILED = {}


def _build_program(mm_dtype_name="float32r", reps=1, skip=()):
    """reps>1 wraps the whole per-core body in a hardware loop (timing).
    skip: subset of {"dve", "pe_bc", "tr"} disabling sections (timing)."""
    import concourse.bacc as bacc
    import concourse.mybir as mybir
    from concourse.tile import TileContext

    f32 = mybir.dt.float32
    mmdt = getattr(mybir.dt, mm_dtype_name)

    nc = bacc.Bacc("TRN2", target_bir_lowering=False, debug=False,
                   num_devices=N_CORES)

    # ---- DRAM tensors (per-core) ----
    xin = nc.dram_tensor("xin", [BPC, LENGTH, HIDDEN], f32,
                         kind="ExternalInput").ap()
    # B weights, phase-folded per chunk: [J, comp(re/im), half, h, p]
    bw = nc.dram_tensor("bw", [HIDDEN, NCH, 2, 2, 128], f32,
                        kind="ExternalInput").ap()
    # C weights, phase-folded per chunk (sign of im folded): [T, comp, half, p, h]
    cw = nc.dram_tensor("cw", [128, NCH, 2, 2, HIDDEN], f32,
                        kind="ExternalInput").ap()
    # in-chunk rotation tables, packed for the 2-mult complex trick:
    # epre/epost: [half, which(0=T1,1=T2), 128, 2, 512]
    epre = nc.dram_tensor("epre", [128, 2, 2, 2, CH], f32,
                          kind="ExternalInput").ap()
    epost = nc.dram_tensor("epost", [128, 2, 2, 2, CH], f32,
                           kind="ExternalInput").ap()
    rcol = nc.dram_tensor("rcol", [128, 2], f32, kind="ExternalInput").ap()
    dw = nc.dram_tensor("dw", [HIDDEN, HIDDEN], f32, kind="ExternalInput").ap()
    eye = nc.dram_tensor("eye", [128, 128], f32, kind="ExternalInput").ap()
    out = nc.dram_tensor("out", [BPC, LENGTH, HIDDEN], f32,
                         kind="ExternalOutput").ap()

    with TileContext(nc) as tc:
        import contextlib

        @contextlib.contextmanager
        def body_loop():
            if reps == 1:
                yield
            else:
                with tc.For_i(0, reps, 1):
                    yield

        with (
            tc.tile_pool(name="const", bufs=1) as cpool,
            tc.tile_pool(name="inat", bufs=4) as inat_pool,
            tc.tile_pool(name="intp", bufs=2) as intr_pool,   # inT per batch
            tc.tile_pool(name="cbuf", bufs=2) as cbuf_pool,   # scan in
            tc.tile_pool(name="ybuf", bufs=2) as ybuf_pool,   # scan out
            tc.tile_pool(name="xbuf", bufs=2) as xbuf_pool,   # post-rot
            tc.tile_pool(name="obuf", bufs=2) as obuf_pool,   # out staging
            tc.tile_pool(name="pst", bufs=1, space="PSUM") as pst,
            tc.tile_pool(name="psb", bufs=1, space="PSUM") as psb,
            tc.tile_pool(name="pso", bufs=2, space="PSUM") as pso,
            tc.tile_pool(name="psq", bufs=1, space="PSUM") as psq,
        ):
            # ---- constants to SBUF ----
            bw_t = cpool.tile([HIDDEN, NCH, 2, 2, 128], mmdt, tag="bw")
            cw_t = cpool.tile([128, NCH, 2, 2, HIDDEN], mmdt, tag="cw")
            epre_t = cpool.tile([128, 2, 2, 2, CH], f32, tag="epre")
            epost_t = cpool.tile([128, 2, 2, 2, CH], f32, tag="epost")
            rcol_t = cpool.tile([128, 2], f32, tag="rcol")
            dw_t = cpool.tile([HIDDEN, HIDDEN], mmdt, tag="dw")
            eye_t = cpool.tile([128, 128], f32, tag="eye")
            nc.sync.dma_start(bw_t[:], bw[:].bitcast(mmdt))
            nc.sync.dma_start(dw_t[:], dw[:].bitcast(mmdt))
            nc.sync.dma_start(cw_t[:], cw[:].bitcast(mmdt))
            for src, dst in [(epre, epre_t),
                             (epost, epost_t), (rcol, rcol_t),
                             (eye, eye_t)]:
                nc.sync.dma_start(dst[:], src[:])

            # broadcast r along free dim for the scan coefficient
            rbc = cpool.tile([128, 2, CH], f32, tag="rbc")
            for half in range(2):
                nc.vector.memset(rbc[:, half], 1.0)
                nc.vector.tensor_scalar_mul(
                    rbc[:, half], rbc[:, half], rcol_t[:, half:half + 1])

            ctx_loop = body_loop()
            ctx_loop.__enter__()
            for b in range(BPC):
                # ---- load + transpose input: inT [h, l] ----
                inT = intr_pool.tile([HIDDEN, LENGTH], mmdt, tag="inT")
                nat = inat_pool.tile([128, LENGTH // 128, HIDDEN], f32,
                                     tag="nat")
                # one 1MB DMA: dst[p, blk, h] = xin[b, 128*blk + p, h]
                nc.sync.dma_start(
                    nat[:],
                    xin[b].rearrange("(n p) h -> p n h", p=128))
                if "tr" not in skip:
                    for q in range(LENGTH // 512):
                        tp = pst.tile([HIDDEN, 4, 128], f32, tag="tp")
                        for j in range(4):
                            nc.tensor.transpose(
                                tp[:, j, :], nat[:, 4 * q + j, :], eye_t[:])
                        nc.scalar.copy(
                            inT[:, 512 * q:512 * (q + 1)],
                            tp[:].rearrange("h f t -> h (f t)"))

                osb = obuf_pool.tile([128, LENGTH // 128, HIDDEN], f32,
                                     tag="osb")
                y_prev = [None, None]
                for J in range(NCH):
                    tsl = slice(CH * J, CH * (J + 1))
                    x_sb = []
                    for half in range(2):
                        # ---- B-proj: bu[p_half, 2, CH] (re|im packed) ----
                        bu = psb.tile([128, 2, CH], f32, tag=f"bu{half}")
                        if "pe_bc" in skip:
                            nc.scalar.memzero(bu[:])
                        else:
                            for comp in range(2):
                                nc.tensor.matmul(
                                    bu[:, comp, :],
                                    bw_t[:, J, comp, half],
                                    inT[:, tsl],
                                    start=True, stop=True)
                        # ---- pre-rotation: c = E- * bu (complex) ----
                        if "dve" not in skip:
                            t1 = xbuf_pool.tile([128, 2, CH], f32, tag="t1")
                            t2 = xbuf_pool.tile([128, 2, CH], f32, tag="t2")
                            cc = cbuf_pool.tile(
                                [128, 2, CH], f32, tag=f"c{half}",
                                name=f"c{half}")
                            nc.vector.tensor_mul(
                                t1[:], bu[:], epre_t[:, half, 0])
                            nc.vector.tensor_mul(
                                t2[:], bu[:], epre_t[:, half, 1])
                            nc.gpsimd.tensor_add(
                                cc[:, 0, :], t1[:, 0, :], t1[:, 1, :])
                            nc.gpsimd.tensor_add(
                                cc[:, 1, :], t2[:, 0, :], t2[:, 1, :])

                        # ---- chained scan: y = r * y_prev + c ----
                        if "dve" not in skip:
                            yy = ybuf_pool.tile(
                                [128, 2, CH], f32, tag=f"y{half}",
                                name=f"y{half}")
                            for comp in range(2):
                                init = (0.0 if y_prev[half] is None else
                                        y_prev[half][:, comp, CH - 1:CH])
                                nc.vector.tensor_tensor_scan(
                                    yy[:, comp, :],
                                    rbc[:, half],
                                    cc[:, comp, :],
                                    init,
                                    op0=mybir.AluOpType.mult,
                                    op1=mybir.AluOpType.add)
                            y_prev[half] = yy

                        # ---- post-rotation: x = E+ * y (complex) ----
                        xs = xbuf_pool.tile([128, 2, CH], mmdt,
                                            tag=f"x{half}", name=f"x{half}")
                        if "dve" in skip:
                            nc.gpsimd.memset(xs[:], 0.0)
                        else:
                            t3 = xbuf_pool.tile([128, 2, CH], f32, tag="t3")
                            t4 = xbuf_pool.tile([128, 2, CH], f32, tag="t4")
                            nc.vector.tensor_mul(
                                t3[:], yy[:], epost_t[:, half, 0])
                            nc.vector.tensor_mul(
                                t4[:], yy[:], epost_t[:, half, 1])
                            nc.gpsimd.tensor_add(
                                xs[:, 0, :], t3[:, 0, :], t3[:, 1, :])
                            nc.gpsimd.tensor_add(
                                xs[:, 1, :], t4[:, 0, :], t4[:, 1, :])
                        x_sb.append(xs)

                    # ---- C-proj + D-residual: outT[h, t] = C'x + D u ----
                    outT = pso.tile([HIDDEN, CH], f32, tag="outT")
                    if "pe_bc" in skip:
                        nc.scalar.memzero(outT[:])
                    else:
                        first = True
                        for comp in range(2):
                            for half in range(2):
                                nc.tensor.matmul(
                                    outT[:],
                                    cw_t[:, J, comp, half],
                                    x_sb[half][:, comp, :],
                                    start=first, stop=False)
                                first = False
                        nc.tensor.matmul(
                            outT[:], dw_t[:], inT[:, tsl],
                            start=False, stop=True)
                    oT = xbuf_pool.tile([HIDDEN, CH], f32, tag="oT")
                    nc.scalar.copy(oT[:], outT[:])
                    tpo = psq.tile([128, 4, HIDDEN], f32, tag="tpo")
                    for i in range(NBLK):
                        nc.tensor.transpose(
                            tpo[:, i, :], oT[:, 128 * i:128 * (i + 1)],
                            eye_t[:])
                    nc.scalar.copy(
                        osb[:, 4 * J:4 * (J + 1), :],
                        tpo[:].rearrange("t f h -> t (f h)").rearrange(
                            "t (f h) -> t f h", h=HIDDEN))
                nc.sync.dma_start(
                    out[b].rearrange("(n p) h -> p n h", p=128), osb[:])

            ctx_loop.__exit__(None, None, None)

    nc.compile()
    return nc


def _host_constants(A_diag, G_diag, steps, B, C, D):
    """Parameter projection + eigenvalues + phase-folded weight tables."""
    A = A_diag.astype(np.float64)
    G = G_diag.astype(np.float64)
    st = steps.astype(np.float64)
    step = 1.0 / (1.0 + np.exp(-st))
    g = np.maximum(G, 0.0)
    denom = np.maximum(step * step, 1e-6)
    s = step * g
    base = np.sqrt(np.maximum(1.0 + s, 1e-6))
    a_low = (2.0 + s - 2.0 * base) / denom
    a_high = (2.0 + s + 2.0 * base) / denom
    a = a_low + np.maximum(A - a_low, 0.0) - np.maximum(A - a_high, 0.0)
    S = 1.0 / (1.0 + step * g)
    T = S + 1.0 - step * step * S * a
    imag = np.sqrt(np.maximum(S - 0.25 * T * T, 0.0))
    lam = 0.5 * T + 1j * imag                      # [P] complex128
    r = np.abs(lam)
    th = np.angle(lam)

    j0 = np.arange(CH, dtype=np.float64)
    # in-chunk rotations, [P, CH]
    cos_m = np.cos(th[:, None] * j0[None, :])
    sin_m = np.sin(th[:, None] * j0[None, :])

    # epre packs for c = exp(-i th t0') * bu:
    #   cre = bur*cos + bui*sin   -> T1 mult table [cos | sin], add halves
    #   cim = bur*(-sin) + bui*cos-> T2 mult table [-sin | cos], add halves
    # epost for x = exp(+i th t0') * y:
    #   xr = yr*cos + yi*(-sin)   -> T1 table [cos | -sin]
    #   xi = yr*sin + yi*cos      -> T2 table [sin | cos]
    epre = np.zeros((128, 2, 2, 2, CH), np.float32)
    epost = np.zeros((128, 2, 2, 2, CH), np.float32)
    for half in range(2):
        psl = slice(128 * half, 128 * (half + 1))
        epre[:, half, 0, 0, :] = cos_m[psl]
        epre[:, half, 0, 1, :] = sin_m[psl]
        epre[:, half, 1, 0, :] = -sin_m[psl]
        epre[:, half, 1, 1, :] = cos_m[psl]
        epost[:, half, 0, 0, :] = cos_m[psl]
        epost[:, half, 0, 1, :] = -sin_m[psl]
        epost[:, half, 1, 0, :] = sin_m[psl]
        epost[:, half, 1, 1, :] = cos_m[psl]

    # chunk-folded B: B_J = exp(-i th * CH * J) * (Br + i Bi)
    Bc = B[..., 0].astype(np.float64) + 1j * B[..., 1].astype(np.float64)
    Cc = C[..., 0].astype(np.float64) + 1j * C[..., 1].astype(np.float64)
    bw = np.zeros((HIDDEN, NCH, 2, 2, 128), np.float32)
    cwt = np.zeros((128, NCH, 2, 2, HIDDEN), np.float32)
    for J in range(NCH):
        ph = np.exp(-1j * th * (CH * J))           # [P]
        BJ = Bc * ph[:, None]                      # [P, H]
        phc = np.exp(+1j * th * (CH * J))
        CT = Cc * phc[None, :]                     # [H, P] (C' = Cre + i Cim)
        for half in range(2):
            psl = slice(128 * half, 128 * (half + 1))
            bw[:, J, 0, half] = BJ.real[psl].T     # lhsT [h, p]
            bw[:, J, 1, half] = BJ.imag[psl].T
            # out = Re{C'_T x} = CTre*xr - CTim*xi ; rhs [p, h]
            cwt[:, J, 0, half] = CT.real[:, psl].T
            cwt[:, J, 1, half] = -CT.imag[:, psl].T

    rcol = np.zeros((128, 2), np.float32)
    rcol[:, 0] = r[:128]
    rcol[:, 1] = r[128:]
    dwm = np.diag(D.astype(np.float64)).astype(np.float32)
    eye = np.eye(128, dtype=np.float32)
    return dict(bw=bw, cw=cwt, epre=epre, epost=epost, rcol=rcol, dw=dwm,
                eye=eye)


def kernel(inputs, A_diag, G_diag, steps, B, C, D):
    from concourse import bass_utils

    inputs = np.asarray(inputs, np.float32)
    consts = _host_constants(np.asarray(A_diag), np.asarray(G_diag),
                             np.asarray(steps), np.asarray(B), np.asarray(C),
                             np.asarray(D))

    if "prog" not in _COMPILED:
        _COMPILED["prog"] = _build_program()
    nc = _COMPILED["prog"]

    in_maps = []
    for core in range(N_CORES):
        m = dict(consts)
        m["xin"] = np.ascontiguousarray(inputs[BPC * core: BPC * (core + 1)])
        in_maps.append(m)
    res = bass_utils.run_bass_kernel_spmd(nc, in_maps,
                                          core_ids=list(range(N_CORES)))
    out = np.concatenate([res.results[i]["out"] for i in range(N_CORES)],
                         axis=0)
    return out.astype(np.float32)



# revision 3
# speedup vs baseline: 1.1059x; 1.1059x over previous
"""DampedLinOSSLayer Trainium2 kernel (8 NeuronCores, batch-sharded).

Math: per SSM channel p, the complex diagonal recurrence
    x_t = lam_p * x_{t-1} + bu_t,   lam_p = r_p * exp(i*th_p)
is factored through the gauge x_t = exp(i*th_p*t) * y_t so the hardware
tensor_tensor_scan (DVE) runs with a REAL per-channel coefficient on the
re/im planes independently.

v2 layout decisions (from HW microbenchmarks):
  - All rotation tables, scan operands, and matmul operands are bf16:
    DVE tensor_tensor gets the 2x perf mode only when every operand is a
    2-byte dtype in SBUF; the scan runs at ~2.09 ns/col in bf16 vs ~3.9
    in f32; bf16 matmuls are 2x faster than f32r below peak p-state.
  - The scan coefficient must be fp32-accurate (a 2^-9 error in r is
    amplified by the ~1/(1-r) memory of the recurrence).  Split
    r = r_hi * r_lo with r_hi exactly representable in bf16 and
    |ln r_lo| <= 2^-9: the scan uses r_hi (exact in bf16), and the
    residual magnitudes r_lo^{-t} / r_lo^{+t} are folded into the
    pre/post rotation tables (bounded by e^4 over t<2048).
  - Rotation tables span the full L=2048 (no chunk phase folding), so B
    and C need only a single weight copy and the scan is one op per
    (half, comp) over the whole sequence: no carry chaining at all.
  - Act engine does every PSUM->SBUF copy (with the f32->bf16 downcast);
    GpSimd cannot touch PSUM and is slow (~2.1 ns/col) but takes the
    post-rotation adds (SBUF bf16) off the critical DVE path.

Per core: 4 batches of the 32.
  - input tiles [l,h] -> PE transpose -> inT [h, l] (bf16)
  - B-proj:  bu[p_half, comp, t] = bw^T @ inT      (PE, bf16)
  - pre-rotation (packed complex mul)              (DVE bf16 2x)
  - scan y = r_hi*y + c along t, full L=2048       (DVE scan, bf16)
  - post-rotation -> x                             (DVE muls + Pool adds)
  - C-proj + D-residual -> out[t, h]               (PE, PSUM-accumulated)
"""

import numpy as np

BATCH, LENGTH, HIDDEN, P = 32, 2048, 128, 256
N_CORES = 8
BPC = BATCH // N_CORES          # batches per core
CH = 512                        # chunk size (PSUM tile granularity)
NCH = LENGTH // CH              # 4 chunks
NBLK = CH // 128                # 4 token-blocks of 128 per chunk

_COMPILED = {}


def _build_program():
    import concourse.bacc as bacc
    import concourse.mybir as mybir
    from concourse.tile import TileContext

    f32 = mybir.dt.float32
    bf16 = mybir.dt.bfloat16

    nc = bacc.Bacc("TRN2", target_bir_lowering=False, debug=False,
                   num_devices=N_CORES)

    # ---- DRAM tensors (per-core) ----
    xin = nc.dram_tensor("xin", [BPC, LENGTH, HIDDEN], f32,
                         kind="ExternalInput").ap()
    # B weights [h, comp, half, p] / C weights [p, comp, half, h] (bf16)
    bw = nc.dram_tensor("bw", [HIDDEN, 2, 2, 128], bf16,
                        kind="ExternalInput").ap()
    cw = nc.dram_tensor("cw", [128, 2, 2, HIDDEN], bf16,
                        kind="ExternalInput").ap()
    # rotation tables [p, half, which, comp, t] over full L (bf16)
    tp = nc.dram_tensor("tp", [128, 2, 2, 2, LENGTH], bf16,
                        kind="ExternalInput").ap()
    tq = nc.dram_tensor("tq", [128, 2, 2, 2, LENGTH], bf16,
                        kind="ExternalInput").ap()
    # scan coefficient r_hi broadcast [p, half, t] (bf16-exact)
    rbc = nc.dram_tensor("rbc", [128, 2, LENGTH], bf16,
                         kind="ExternalInput").ap()
    dw = nc.dram_tensor("dw", [HIDDEN, HIDDEN], bf16,
                        kind="ExternalInput").ap()
    eye32 = nc.dram_tensor("eye32", [128, 128], f32,
                           kind="ExternalInput").ap()
    eye16 = nc.dram_tensor("eye16", [128, 128], bf16,
                           kind="ExternalInput").ap()
    out = nc.dram_tensor("out", [BPC, LENGTH, HIDDEN], f32,
                         kind="ExternalOutput").ap()

    with TileContext(nc) as tc:
        with (
            tc.tile_pool(name="const", bufs=1) as cpool,
            tc.tile_pool(name="inat", bufs=2) as inat_pool,
            tc.tile_pool(name="intp", bufs=2) as intr_pool,   # inT per batch
            tc.tile_pool(name="busb", bufs=2) as busb_pool,   # bu bf16 chunk
            tc.tile_pool(name="t12", bufs=2) as t12_pool,     # mul scratch
            tc.tile_pool(name="cbuf", bufs=2) as cbuf_pool,   # scan in (full L)
            tc.tile_pool(name="ybuf", bufs=1) as ybuf_pool,   # scan out (full L)
            tc.tile_pool(name="t34", bufs=2) as t34_pool,     # post scratch
            tc.tile_pool(name="xbuf", bufs=2) as xbuf_pool,   # post-rot chunk
            tc.tile_pool(name="otb", bufs=2) as otb_pool,     # outT staging
            tc.tile_pool(name="obuf", bufs=2) as obuf_pool,   # out staging
            tc.tile_pool(name="pst", bufs=1, space="PSUM") as pst,
            tc.tile_pool(name="psb", bufs=2, space="PSUM") as psb,
            tc.tile_pool(name="pso", bufs=1, space="PSUM") as pso,
            tc.tile_pool(name="psq", bufs=1, space="PSUM") as psq,
        ):
            # ---- constants to SBUF ----
            bw_t = cpool.tile([HIDDEN, 2, 2, 128], bf16, tag="bw")
            cw_t = cpool.tile([128, 2, 2, HIDDEN], bf16, tag="cw")
            tp_t = cpool.tile([128, 2, 2, 2, LENGTH], bf16, tag="tp")
            tq_t = cpool.tile([128, 2, 2, 2, LENGTH], bf16, tag="tq")
            rbc_t = cpool.tile([128, 2, LENGTH], bf16, tag="rbc")
            dw_t = cpool.tile([HIDDEN, HIDDEN], bf16, tag="dw")
            eye32_t = cpool.tile([128, 128], f32, tag="eye32")
            eye16_t = cpool.tile([128, 128], bf16, tag="eye16")
            # order matters: first consumers first
            nc.sync.dma_start(eye32_t[:], eye32[:])
            nc.sync.dma_start(bw_t[:], bw[:])
            nc.sync.dma_start(tp_t[:], tp[:])
            nc.sync.dma_start(rbc_t[:], rbc[:])
            nc.sync.dma_start(tq_t[:], tq[:])
            nc.sync.dma_start(cw_t[:], cw[:])
            nc.sync.dma_start(dw_t[:], dw[:])
            nc.sync.dma_start(eye16_t[:], eye16[:])

            for b in range(BPC):
                # ---- load + transpose input: inT [h, l] bf16 ----
                inT = intr_pool.tile([HIDDEN, LENGTH], bf16, tag="inT")
                nat = inat_pool.tile([128, LENGTH // 128, HIDDEN], f32,
                                     tag="nat")
                nc.sync.dma_start(
                    nat[:], xin[b].rearrange("(n p) h -> p n h", p=128))
                for q in range(NCH):
                    tpp = pst.tile([HIDDEN, 4, 128], f32, tag="tpp")
                    for j in range(4):
                        nc.tensor.transpose(
                            tpp[:, j, :], nat[:, 4 * q + j, :], eye32_t[:])
                    nc.scalar.copy(
                        inT[:, CH * q:CH * (q + 1)],
                        tpp[:].rearrange("h f t -> h (f t)"))

                # ---- B-proj + pre-rotation, chunk by chunk -> cc full L ----
                cc = cbuf_pool.tile([128, 2, 2, LENGTH], bf16, tag="cc",
                                    name=f"cc{b}")
                for J in range(NCH):
                    tsl = slice(CH * J, CH * (J + 1))
                    for half in range(2):
                        bu = psb.tile([128, 2, CH], f32, tag="bu")
                        for comp in range(2):
                            nc.tensor.matmul(
                                bu[:, comp, :],
                                bw_t[:, comp, half],
                                inT[:, tsl],
                                start=True, stop=True)
                        busb = busb_pool.tile([128, 2, CH], bf16, tag="busb")
                        nc.scalar.copy(busb[:], bu[:])
                        t12 = t12_pool.tile([128, 2, 2, CH], bf16, tag="t12")
                        for w in range(2):
                            nc.vector.tensor_mul(
                                t12[:, w], busb[:], tp_t[:, half, w, :, tsl])
                        # cc[half, comp] = t12[w, 0] + t12[w, 1] -> packed add
                        nc.vector.tensor_add(
                            cc[:, half, :, tsl],
                            t12[:, :, 0, :], t12[:, :, 1, :])

                # ---- scans: y = r_hi * y + c, full L per (half, comp) ----
                yy = ybuf_pool.tile([128, 2, 2, LENGTH], bf16, tag="yy",
                                    name=f"yy{b}")
                for half in range(2):
                    for comp in range(2):
                        nc.vector.tensor_tensor_scan(
                            yy[:, half, comp, :],
                            rbc_t[:, half],
                            cc[:, half, comp, :],
                            0.0,
                            op0=mybir.AluOpType.mult,
                            op1=mybir.AluOpType.add)

                # ---- post-rotation + C-proj + D-residual, per chunk ----
                osb = obuf_pool.tile([128, LENGTH // 128, HIDDEN], f32,
                                     tag="osb")
                for J in range(NCH):
                    tsl = slice(CH * J, CH * (J + 1))
                    xs = xbuf_pool.tile([128, 2, 2, CH], bf16, tag="xs",
                                        name=f"xs{b}_{J}")
                    for half in range(2):
                        t34 = t34_pool.tile([128, 2, 2, CH], bf16, tag="t34")
                        for w in range(2):
                            nc.vector.tensor_mul(
                                t34[:, w], yy[:, half, :, tsl],
                                tq_t[:, half, w, :, tsl])
                        # post adds on Pool (SBUF bf16 only)
                        nc.gpsimd.tensor_add(
                            xs[:, half],
                            t34[:, :, 0, :], t34[:, :, 1, :])

                    outT = pso.tile([HIDDEN, CH], f32, tag="outT")
                    first = True
                    for comp in range(2):
                        for half in range(2):
                            nc.tensor.matmul(
                                outT[:],
                                cw_t[:, comp, half],
                                xs[:, half, comp, :],
                                start=first, stop=False)
                            first = False
                    nc.tensor.matmul(
                        outT[:], dw_t[:], inT[:, tsl],
                        start=False, stop=True)
                    oT = otb_pool.tile([HIDDEN, CH], bf16, tag="oT")
                    nc.scalar.copy(oT[:], outT[:])
                    tpo = psq.tile([128, 4, HIDDEN], bf16, tag="tpo")
                    for i in range(NBLK):
                        nc.tensor.transpose(
                            tpo[:, i, :], oT[:, 128 * i:128 * (i + 1)],
                            eye16_t[:])
                    nc.scalar.copy(
                        osb[:, 4 * J:4 * (J + 1), :],
                        tpo[:].rearrange("t f h -> t (f h)").rearrange(
                            "t (f h) -> t f h", h=HIDDEN))
                nc.sync.dma_start(
                    out[b].rearrange("(n p) h -> p n h", p=128), osb[:])

    nc.compile()
    return nc


def _host_constants(A_diag, G_diag, steps, B, C, D):
    """Parameter projection + eigenvalues + full-length rotation tables."""
    import ml_dtypes

    A = A_diag.astype(np.float64)
    G = G_diag.astype(np.float64)
    st = steps.astype(np.float64)
    step = 1.0 / (1.0 + np.exp(-st))
    g = np.maximum(G, 0.0)
    denom = np.maximum(step * step, 1e-6)
    s = step * g
    base = np.sqrt(np.maximum(1.0 + s, 1e-6))
    a_low = (2.0 + s - 2.0 * base) / denom
    a_high = (2.0 + s + 2.0 * base) / denom
    a = a_low + np.maximum(A - a_low, 0.0) - np.maximum(A - a_high, 0.0)
    S = 1.0 / (1.0 + step * g)
    T = S + 1.0 - step * step * S * a
    imag = np.sqrt(np.maximum(S - 0.25 * T * T, 0.0))
    lam = 0.5 * T + 1j * imag                      # [P] complex128
    r = np.abs(lam)
    th = np.angle(lam)

    # r = r_hi * r_lo with r_hi bf16-exact; fold r_lo^{+-t} into the tables
    r_hi = np.asarray(r.astype(np.float32), dtype=ml_dtypes.bfloat16)
    r_hi64 = r_hi.astype(np.float64)
    log_rlo = np.log(r) - np.log(r_hi64)           # |.| <= ~2^-9

    t = np.arange(LENGTH, dtype=np.float64)
    bf = ml_dtypes.bfloat16

    tp = np.zeros((128, 2, 2, 2, LENGTH), np.float32)
    tq = np.zeros((128, 2, 2, 2, LENGTH), np.float32)
    rbc = np.zeros((128, 2, LENGTH), np.float32)
    for half in range(2):
        psl = slice(128 * half, 128 * (half + 1))
        ang = th[psl, None] * t[None, :]
        cos_m = np.cos(ang)
        sin_m = np.sin(ang)
        mag_pre = np.exp(-log_rlo[psl, None] * t[None, :])   # r_lo^{-t}
        mag_post = np.exp(+log_rlo[psl, None] * t[None, :])  # r_lo^{+t}
        # pre: c = exp(-i th t) * r_lo^{-t} * bu
        tp[:, half, 0, 0, :] = cos_m * mag_pre
        tp[:, half, 0, 1, :] = sin_m * mag_pre
        tp[:, half, 1, 0, :] = -sin_m * mag_pre
        tp[:, half, 1, 1, :] = cos_m * mag_pre
        # post: x = exp(+i th t) * r_lo^{+t} * y
        tq[:, half, 0, 0, :] = cos_m * mag_post
        tq[:, half, 0, 1, :] = -sin_m * mag_post
        tq[:, half, 1, 0, :] = sin_m * mag_post
        tq[:, half, 1, 1, :] = cos_m * mag_post
        rbc[:, half, :] = r_hi64[psl, None]

    Br = B[..., 0].astype(np.float64)
    Bi = B[..., 1].astype(np.float64)
    Cr = C[..., 0].astype(np.float64)
    Ci = C[..., 1].astype(np.float64)
    bw = np.zeros((HIDDEN, 2, 2, 128), np.float32)
    cw = np.zeros((128, 2, 2, HIDDEN), np.float32)
    for half in range(2):
        psl = slice(128 * half, 128 * (half + 1))
        bw[:, 0, half] = Br[psl].T                 # lhsT [h, p]
        bw[:, 1, half] = Bi[psl].T
        cw[:, 0, half] = Cr[:, psl].T              # out = Cr*xr - Ci*xi
        cw[:, 1, half] = -Ci[:, psl].T

    dwm = np.diag(D.astype(np.float64)).astype(np.float32)
    return dict(
        bw=bw.astype(bf), cw=cw.astype(bf),
        tp=tp.astype(bf), tq=tq.astype(bf),
        rbc=rbc.astype(bf), dw=dwm.astype(bf),
        eye32=np.eye(128, dtype=np.float32),
        eye16=np.eye(128, dtype=np.float32).astype(bf),
    )


def kernel(inputs, A_diag, G_diag, steps, B, C, D):
    from concourse import bass_utils

    inputs = np.asarray(inputs, np.float32)
    consts = _host_constants(np.asarray(A_diag), np.asarray(G_diag),
                             np.asarray(steps), np.asarray(B), np.asarray(C),
                             np.asarray(D))

    if "prog" not in _COMPILED:
        _COMPILED["prog"] = _build_program()
    nc = _COMPILED["prog"]

    in_maps = []
    for core in range(N_CORES):
        m = dict(consts)
        m["xin"] = np.ascontiguousarray(inputs[BPC * core: BPC * (core + 1)])
        in_maps.append(m)
    res = bass_utils.run_bass_kernel_spmd(nc, in_maps,
                                          core_ids=list(range(N_CORES)))
    out = np.concatenate([res.results[i]["out"] for i in range(N_CORES)],
                         axis=0)
    return out.astype(np.float32)


# revision 15
# speedup vs baseline: 1.2043x; 1.0890x over previous
"""DampedLinOSSLayer Trainium2 kernel (8 NeuronCores, batch-sharded).

Math: per SSM channel p, the complex diagonal recurrence
    x_t = lam_p * x_{t-1} + bu_t,   lam_p = r_p * exp(i*th_p)
is factored through the gauge x_t = exp(i*th_p*t) * y_t so the hardware
tensor_tensor_scan (DVE) runs with a REAL per-channel coefficient on the
re/im planes independently.

v2 layout decisions (from HW microbenchmarks):
  - All rotation tables, scan operands, and matmul operands are bf16:
    DVE tensor_tensor gets the 2x perf mode only when every operand is a
    2-byte dtype in SBUF; the scan runs at ~2.09 ns/col in bf16 vs ~3.9
    in f32; bf16 matmuls are 2x faster than f32r below peak p-state.
  - The scan coefficient must be fp32-accurate (a 2^-9 error in r is
    amplified by the ~1/(1-r) memory of the recurrence).  Split
    r = r_hi * r_lo with r_hi exactly representable in bf16 and
    |ln r_lo| <= 2^-9: the scan uses r_hi (exact in bf16), and the
    residual magnitudes r_lo^{-t} / r_lo^{+t} are folded into the
    pre/post rotation tables (bounded by e^4 over t<2048).
  - Rotation tables span the full L=2048 (no chunk phase folding), so B
    and C need only a single weight copy and the scan is one op per
    (half, comp) over the whole sequence: no carry chaining at all.
  - Act engine does every PSUM->SBUF copy (with the f32->bf16 downcast);
    GpSimd cannot touch PSUM and is slow (~2.1 ns/col) but takes the
    post-rotation adds (SBUF bf16) off the critical DVE path.

Per core: 4 batches of the 32.
  - input tiles [l,h] -> PE transpose -> inT [h, l] (bf16)
  - B-proj:  bu[p_half, comp, t] = bw^T @ inT      (PE, bf16)
  - pre-rotation (packed complex mul)              (DVE bf16 2x)
  - scan y = r_hi*y + c along t, full L=2048       (DVE scan, bf16)
  - post-rotation -> x                             (DVE muls + Pool adds)
  - C-proj + D-residual -> out[t, h]               (PE, PSUM-accumulated)
"""

import numpy as np

BATCH, LENGTH, HIDDEN, P = 32, 2048, 128, 256
N_CORES = 8
BPC = BATCH // N_CORES          # batches per core
CH = 512                        # chunk size (PSUM tile granularity)
NCH = LENGTH // CH              # 4 chunks
NBLK = CH // 128                # 4 token-blocks of 128 per chunk

_COMPILED = {}


def _build_program():
    import concourse.bacc as bacc
    import concourse.mybir as mybir
    from concourse.ap import AP
    from concourse.tile import TileContext

    def dup2(a):
        """Insert a stride-0 'which' dim after the partition dim."""
        return AP(a.tensor, a.offset, [list(a.ap[0]), [0, 2],
                                       *[list(x) for x in a.ap[1:]]])

    f32 = mybir.dt.float32
    bf16 = mybir.dt.bfloat16

    nc = bacc.Bacc("TRN2", target_bir_lowering=False, debug=False,
                   num_devices=N_CORES)

    # ---- DRAM tensors (per-core) ----
    xin = nc.dram_tensor("xin", [BPC, LENGTH, HIDDEN], f32,
                         kind="ExternalInput").ap()
    # B weights [h, comp, half, p] / C weights [p, comp, half, h] (bf16)
    bw = nc.dram_tensor("bw", [HIDDEN, 2, 2, 128], bf16,
                        kind="ExternalInput").ap()
    cw = nc.dram_tensor("cw", [128, 2, 2, HIDDEN], bf16,
                        kind="ExternalInput").ap()
    # rotation tables [p, half, which, comp, t] over full L (bf16)
    tp = nc.dram_tensor("tp", [128, 2, 2, 2, LENGTH], bf16,
                        kind="ExternalInput").ap()
    tq = nc.dram_tensor("tq", [128, 2, 2, 2, LENGTH], bf16,
                        kind="ExternalInput").ap()
    # scan coefficient r_hi broadcast [p, half, t] (bf16-exact)
    rbc = nc.dram_tensor("rbc", [128, 2, LENGTH], bf16,
                         kind="ExternalInput").ap()
    dw = nc.dram_tensor("dw", [HIDDEN, HIDDEN], bf16,
                        kind="ExternalInput").ap()
    eye32 = nc.dram_tensor("eye32", [128, 128], f32,
                           kind="ExternalInput").ap()
    eye16 = nc.dram_tensor("eye16", [128, 128], bf16,
                           kind="ExternalInput").ap()
    out = nc.dram_tensor("out", [BPC, LENGTH, HIDDEN], f32,
                         kind="ExternalOutput").ap()

    with TileContext(nc) as tc:
        with (
            tc.tile_pool(name="const", bufs=1) as cpool,
            tc.tile_pool(name="inat", bufs=2) as inat_pool,
            tc.tile_pool(name="intp", bufs=2) as intr_pool,   # inT per batch
            tc.tile_pool(name="busb", bufs=2) as busb_pool,   # bu bf16 chunk
            tc.tile_pool(name="t12", bufs=2) as t12_pool,     # mul scratch
            tc.tile_pool(name="cbuf", bufs=2) as cbuf_pool,   # scan in (full L)
            tc.tile_pool(name="ybuf", bufs=1) as ybuf_pool,   # scan out (full L)
            tc.tile_pool(name="t34", bufs=2) as t34_pool,     # post scratch
            tc.tile_pool(name="xbuf", bufs=2) as xbuf_pool,   # post-rot chunk
            tc.tile_pool(name="otb", bufs=2) as otb_pool,     # outT staging
            tc.tile_pool(name="obuf", bufs=2) as obuf_pool,   # out staging
            tc.tile_pool(name="pst", bufs=1, space="PSUM") as pst,
            tc.tile_pool(name="psb", bufs=2, space="PSUM") as psb,
            tc.tile_pool(name="pso", bufs=1, space="PSUM") as pso,
            tc.tile_pool(name="psq", bufs=1, space="PSUM") as psq,
        ):
            # ---- constants to SBUF ----
            bw_t = cpool.tile([HIDDEN, 2, 2, 128], bf16, tag="bw")
            cw_t = cpool.tile([128, 2, 2, HIDDEN], bf16, tag="cw")
            tp_t = cpool.tile([128, 2, 2, 2, LENGTH], bf16, tag="tp")
            tq_t = cpool.tile([128, 2, 2, 2, LENGTH], bf16, tag="tq")
            rbc_t = cpool.tile([128, 2, LENGTH], bf16, tag="rbc")
            dw_t = cpool.tile([HIDDEN, HIDDEN], bf16, tag="dw")
            eye32_t = cpool.tile([128, 128], f32, tag="eye32")
            eye16_t = cpool.tile([128, 128], bf16, tag="eye16")
            # order matters: first consumers first.  tq/cw/dw/eye16 are
            # deferred into the batch-0 body so their DMAs don't contend
            # with batch-0's table reads on the SBUF ports.
            nc.sync.dma_start(eye32_t[:], eye32[:])
            nc.sync.dma_start(eye16_t[:], eye16[:])
            nc.sync.dma_start(bw_t[:], bw[:])
            nc.sync.dma_start(tp_t[:], tp[:])
            nc.sync.dma_start(rbc_t[:], rbc[:])

            for b in range(BPC):
                # ---- load + cast + transpose input: inT [h, l] bf16 ----
                inT = intr_pool.tile([HIDDEN, LENGTH], bf16, tag="inT")
                nat = inat_pool.tile([128, LENGTH // 128, HIDDEN], f32,
                                     tag="nat")
                nat16 = inat_pool.tile([128, LENGTH // 128, HIDDEN], bf16,
                                       tag="nat16")
                nc.sync.dma_start(
                    nat[:], xin[b].rearrange("(n p) h -> p n h", p=128))
                nc.scalar.copy(nat16[:], nat[:])
                for q in range(NCH):
                    tpp = pst.tile([HIDDEN, 4, 128], bf16, tag="tpp")
                    for j in range(4):
                        nc.tensor.transpose(
                            tpp[:, j, :], nat16[:, 4 * q + j, :], eye16_t[:])
                    nc.scalar.copy(
                        inT[:, CH * q:CH * (q + 1)],
                        tpp[:].rearrange("h f t -> h (f t)"))

                # ---- B-proj + pre-rotation, chunk by chunk -> cc full L ----
                cc = cbuf_pool.tile([128, 2, 2, LENGTH], bf16, tag="cc",
                                    name=f"cc{b}")
                for J in range(NCH):
                    tsl = slice(CH * J, CH * (J + 1))
                    for half in range(2):
                        bu = psb.tile([128, 2, CH], f32, tag="bu")
                        for comp in range(2):
                            nc.tensor.matmul(
                                bu[:, comp, :],
                                bw_t[:, comp, half],
                                inT[:, tsl],
                                start=True, stop=True)
                        busb = busb_pool.tile([128, 2, CH], bf16, tag="busb")
                        nc.scalar.copy(busb[:], bu[:])
                        t12 = t12_pool.tile([128, 2, 2, CH], bf16, tag="t12")
                        # one mul op: busb broadcast (stride-0) over `which`
                        nc.vector.tensor_mul(
                            t12[:], dup2(busb[:]), tp_t[:, half, :, :, tsl])
                        # cc[half, comp] = t12[w, 0] + t12[w, 1] -> packed add
                        nc.vector.tensor_add(
                            cc[:, half, :, tsl],
                            t12[:, :, 0, :], t12[:, :, 1, :])

                if b == 0:
                    # deferred constant DMAs: queued behind batch-0 input
                    # work so they don't contend with batch-0 table reads
                    nc.sync.dma_start(tq_t[:], tq[:])
                    nc.sync.dma_start(cw_t[:], cw[:])
                    nc.sync.dma_start(dw_t[:], dw[:])

                # ---- scans: y = r_hi * y + c, full L per (half, comp) ----
                yy = ybuf_pool.tile([128, 2, 2, LENGTH], bf16, tag="yy",
                                    name=f"yy{b}")
                for half in range(2):
                    for comp in range(2):
                        nc.vector.tensor_tensor_scan(
                            yy[:, half, comp, :],
                            rbc_t[:, half],
                            cc[:, half, comp, :],
                            0.0,
                            op0=mybir.AluOpType.mult,
                            op1=mybir.AluOpType.add)

                # ---- post-rotation + C-proj + D-residual, per chunk ----
                osb = obuf_pool.tile([128, LENGTH // 128, HIDDEN], f32,
                                     tag="osb")
                for J in range(NCH):
                    tsl = slice(CH * J, CH * (J + 1))
                    xs = xbuf_pool.tile([128, 2, 2, CH], bf16, tag="xs",
                                        name=f"xs{b}_{J}")
                    for half in range(2):
                        t34 = t34_pool.tile([128, 2, 2, CH], bf16, tag="t34")
                        nc.vector.tensor_mul(
                            t34[:], dup2(yy[:, half, :, tsl]),
                            tq_t[:, half, :, :, tsl])
                        # post adds on Pool (SBUF bf16 only)
                        nc.gpsimd.tensor_add(
                            xs[:, half],
                            t34[:, :, 0, :], t34[:, :, 1, :])

                    outT = pso.tile([HIDDEN, CH], f32, tag="outT")
                    first = True
                    for comp in range(2):
                        for half in range(2):
                            nc.tensor.matmul(
                                outT[:],
                                cw_t[:, comp, half],
                                xs[:, half, comp, :],
                                start=first, stop=False)
                            first = False
                    nc.tensor.matmul(
                        outT[:], dw_t[:], inT[:, tsl],
                        start=False, stop=True)
                    oT = otb_pool.tile([HIDDEN, CH], bf16, tag="oT")
                    nc.scalar.copy(oT[:], outT[:])
                    tpo = psq.tile([128, 4, HIDDEN], bf16, tag="tpo")
                    for i in range(NBLK):
                        nc.tensor.transpose(
                            tpo[:, i, :], oT[:, 128 * i:128 * (i + 1)],
                            eye16_t[:])
                    nc.scalar.copy(
                        osb[:, 4 * J:4 * (J + 1), :],
                        tpo[:].rearrange("t f h -> t (f h)").rearrange(
                            "t (f h) -> t f h", h=HIDDEN))
                nc.sync.dma_start(
                    out[b].rearrange("(n p) h -> p n h", p=128), osb[:])

    nc.compile()
    return nc


def _host_constants(A_diag, G_diag, steps, B, C, D):
    """Parameter projection + eigenvalues + full-length rotation tables."""
    import ml_dtypes

    A = A_diag.astype(np.float64)
    G = G_diag.astype(np.float64)
    st = steps.astype(np.float64)
    step = 1.0 / (1.0 + np.exp(-st))
    g = np.maximum(G, 0.0)
    denom = np.maximum(step * step, 1e-6)
    s = step * g
    base = np.sqrt(np.maximum(1.0 + s, 1e-6))
    a_low = (2.0 + s - 2.0 * base) / denom
    a_high = (2.0 + s + 2.0 * base) / denom
    a = a_low + np.maximum(A - a_low, 0.0) - np.maximum(A - a_high, 0.0)
    S = 1.0 / (1.0 + step * g)
    T = S + 1.0 - step * step * S * a
    imag = np.sqrt(np.maximum(S - 0.25 * T * T, 0.0))
    lam = 0.5 * T + 1j * imag                      # [P] complex128
    r = np.abs(lam)
    th = np.angle(lam)

    # r = r_hi * r_lo with r_hi bf16-exact; fold r_lo^{+-t} into the tables
    r_hi = np.asarray(r.astype(np.float32), dtype=ml_dtypes.bfloat16)
    r_hi64 = r_hi.astype(np.float64)
    log_rlo = np.log(r) - np.log(r_hi64)           # |.| <= ~2^-9

    t = np.arange(LENGTH, dtype=np.float64)
    bf = ml_dtypes.bfloat16

    tp = np.zeros((128, 2, 2, 2, LENGTH), np.float32)
    tq = np.zeros((128, 2, 2, 2, LENGTH), np.float32)
    rbc = np.zeros((128, 2, LENGTH), np.float32)
    for half in range(2):
        psl = slice(128 * half, 128 * (half + 1))
        ang = th[psl, None] * t[None, :]
        cos_m = np.cos(ang)
        sin_m = np.sin(ang)
        mag_pre = np.exp(-log_rlo[psl, None] * t[None, :])   # r_lo^{-t}
        mag_post = np.exp(+log_rlo[psl, None] * t[None, :])  # r_lo^{+t}
        # pre: c = exp(-i th t) * r_lo^{-t} * bu
        tp[:, half, 0, 0, :] = cos_m * mag_pre
        tp[:, half, 0, 1, :] = sin_m * mag_pre
        tp[:, half, 1, 0, :] = -sin_m * mag_pre
        tp[:, half, 1, 1, :] = cos_m * mag_pre
        # post: x = exp(+i th t) * r_lo^{+t} * y
        tq[:, half, 0, 0, :] = cos_m * mag_post
        tq[:, half, 0, 1, :] = -sin_m * mag_post
        tq[:, half, 1, 0, :] = sin_m * mag_post
        tq[:, half, 1, 1, :] = cos_m * mag_post
        rbc[:, half, :] = r_hi64[psl, None]

    Br = B[..., 0].astype(np.float64)
    Bi = B[..., 1].astype(np.float64)
    Cr = C[..., 0].astype(np.float64)
    Ci = C[..., 1].astype(np.float64)
    bw = np.zeros((HIDDEN, 2, 2, 128), np.float32)
    cw = np.zeros((128, 2, 2, HIDDEN), np.float32)
    for half in range(2):
        psl = slice(128 * half, 128 * (half + 1))
        bw[:, 0, half] = Br[psl].T                 # lhsT [h, p]
        bw[:, 1, half] = Bi[psl].T
        cw[:, 0, half] = Cr[:, psl].T              # out = Cr*xr - Ci*xi
        cw[:, 1, half] = -Ci[:, psl].T

    dwm = np.diag(D.astype(np.float64)).astype(np.float32)
    return dict(
        bw=bw.astype(bf), cw=cw.astype(bf),
        tp=tp.astype(bf), tq=tq.astype(bf),
        rbc=rbc.astype(bf), dw=dwm.astype(bf),
        eye32=np.eye(128, dtype=np.float32),
        eye16=np.eye(128, dtype=np.float32).astype(bf),
    )


def kernel(inputs, A_diag, G_diag, steps, B, C, D):
    from concourse import bass_utils

    inputs = np.asarray(inputs, np.float32)
    consts = _host_constants(np.asarray(A_diag), np.asarray(G_diag),
                             np.asarray(steps), np.asarray(B), np.asarray(C),
                             np.asarray(D))

    if "prog" not in _COMPILED:
        _COMPILED["prog"] = _build_program()
    nc = _COMPILED["prog"]

    in_maps = []
    for core in range(N_CORES):
        m = dict(consts)
        m["xin"] = np.ascontiguousarray(inputs[BPC * core: BPC * (core + 1)])
        in_maps.append(m)
    res = bass_utils.run_bass_kernel_spmd(nc, in_maps,
                                          core_ids=list(range(N_CORES)))
    out = np.concatenate([res.results[i]["out"] for i in range(N_CORES)],
                         axis=0)
    return out.astype(np.float32)
